# revision 1
# baseline (speedup 1.0000x reference)
"""AutoCorrelationLoss Trainium2 kernel (8-core SPMD, data-parallel over batch).

Math: for each row x (length L=8192), with com = L - 128 = 8064 = 63*128:
  ac[k] = mean(x0c * (Y_k - mean(Y_k)))  where x0c = x[:com] - mean(x[:com])
Since sum(x0c) = 0, the mean(Y_k) term vanishes:
  com * ac[k] = c[k] = sum_j x0c[j] * x[j+k]
Decompose j = 128*t + p (t<63, p<128) and let T[t, f] = x[128t + f] (f<256),
Tc = T[:, :128] - mean(x[:com]).  Then with H = Tc.T @ T  ([128, 256]):
  c[k] = sum_j H[j, j+k]   (a diagonal/skew sum, k = 0..128)
r[k] = ac[k]/ac[0] = c[k]/c[0];  loss = mean_{b,k} |r_fake - r_real|.

Per core: 4 batch rows x {fake, real} = 8 row-tensors. One fp32 matmul
[63,128]x[63,256] per row-tensor, a diagonal-stride DMA de-skews H so the
129 skew-sums become column sums done by a ones-matmul.
"""

import os
import sys

sys.path.insert(0, "/opt/trn_rl_repo")

import numpy as np

import concourse.bacc as bacc
import concourse.bass as bass
import concourse.mybir as mybir
import concourse.tile as tile
from concourse.bass_utils import run_bass_kernel_spmd
from concourse.tile_rust import add_dep_helper

B, L = 32, 8192
NCOEF = 128            # lags 0..128 -> 129 values
COM = L - NCOEF        # 8064 = 63 * 128
NT = 63                # contraction chunks
HALO = 256             # halo width per chunk
NK = NCOEF + 1         # 129
N_CORES = 8
ROWS_PER_CORE = B // N_CORES      # 4 batch rows per core
RT = 2 * ROWS_PER_CORE            # 8 row-tensors (fake rows then real rows)

FP32 = mybir.dt.float32


def build_program(debug_taps=False, reps=1, stop_after="full"):
    # stop_after: "loads" | "matmul" | "hsb" | "hd" | "diag" | "reduce" | "full"
    STAGES = ["loads", "matmul", "hsb", "hd", "diag", "reduce", "full"]
    lvl = STAGES.index(stop_after)
    nc = bacc.Bacc(
        "TRN2",
        target_bir_lowering=False,
        debug=False,
        num_devices=N_CORES,
    )

    xin = nc.dram_tensor("xin", (RT, L), FP32, kind="ExternalInput")
    out = nc.dram_tensor("out", (ROWS_PER_CORE, 1), FP32, kind="ExternalOutput")
    if debug_taps:
        hdram = nc.dram_tensor("hdram", (RT, 128, HALO), FP32,
                               kind="ExternalOutput")
        rdram = nc.dram_tensor("rdram", (128, RT * NK), FP32,
                               kind="ExternalOutput")
        cdram = nc.dram_tensor("cdram", (ROWS_PER_CORE, 2 * NK), FP32,
                               kind="ExternalOutput")

    with tile.TileContext(nc) as tc:
        with (
            tc.tile_pool(name="tpool", bufs=3) as tpool,
            tc.tile_pool(name="wpool", bufs=3) as wpool,
            tc.tile_pool(name="spool", bufs=4) as spool,
            tc.tile_pool(name="hsb", bufs=3) as hsbpool,
            tc.tile_pool(name="persist", bufs=1) as persist,
            tc.tile_pool(name="hd", bufs=RT, space=bass.MemorySpace.DRAM) as hdpool,
            tc.tile_pool(name="hps", bufs=4, space=bass.MemorySpace.PSUM) as hps,
            tc.tile_pool(name="bps", bufs=2, space=bass.MemorySpace.PSUM) as bps,
            tc.tile_pool(name="cps", bufs=2, space=bass.MemorySpace.PSUM) as cps,
        ):
            ones63 = persist.tile([NT, NT], FP32)
            nc.vector.memset(ones63[:], 1.0)
            ones128 = persist.tile([128, 1], FP32)
            nc.vector.memset(ones128[:], 1.0)
            # de-skewed diagonals for all 8 row-tensors, side by side
            rbig = persist.tile([128, RT * NK], FP32)

            def emit_rep():
              diag_reads = []
              ka = None
              for rt in range(RT):
                # --- contiguous loads: XA[t] = x[128t : 128t+128] (t<64),
                #     XB[t] = x[128(t+1) : 128(t+2)] (t<63)
                xa = tpool.tile([NT + 1, 128], FP32, tag="xa")
                nc.sync.dma_start(
                    xa[:], bass.AP(xin, rt * L, [[128, NT + 1], [1, 128]]))
                xb = tpool.tile([NT, 128], FP32, tag="xb")
                nc.sync.dma_start(
                    xb[:], bass.AP(xin, rt * L + 128, [[128, NT], [1, 128]]))
                if lvl == 0:
                    ka = xa
                    continue

                # --- mean of x[0:com]
                rowsum = spool.tile([NT, 1], FP32, tag="rowsum")
                nc.vector.tensor_reduce(
                    rowsum[:], xa[0:NT, :], mybir.AxisListType.X,
                    mybir.AluOpType.add,
                )
                # broadcast total over 63 partitions: ones63.T @ rowsum
                bcast = bps.tile([NT, 1], FP32, tag="bcast")
                nc.tensor.matmul(bcast[:], ones63[:], rowsum[:],
                                 start=True, stop=True)
                m0 = spool.tile([NT, 1], FP32, tag="m0")
                nc.scalar.mul(m0[:], bcast[:], 1.0 / COM)

                # --- centered stationary operand
                tc_tile = wpool.tile([NT, 128], FP32, tag="w")
                nc.vector.tensor_scalar_sub(tc_tile[:], xa[0:NT, :], m0[:])

                # --- H = Tc.T @ [XA | XB]  -> PSUM [128, 256]
                h_ps = hps.tile([128, HALO], FP32, tag="h")
                nc.tensor.matmul(h_ps[:, 0:128], tc_tile[:], xa[0:NT, :],
                                 start=True, stop=True)
                nc.tensor.matmul(h_ps[:, 128:HALO], tc_tile[:], xb[:],
                                 start=True, stop=True)
                if lvl == 1:
                    kat = spool.tile([1, 1], FP32, tag="ka")
                    nc.scalar.copy(kat[:], h_ps[0:1, 0:1])
                    ka = kat
                    continue

                # --- PSUM -> SBUF (alternate engines to balance load)
                h_sb = hsbpool.tile([128, HALO], FP32, tag="hsb")
                if rt % 2 == 0:
                    nc.vector.tensor_copy(h_sb[:], h_ps[:])
                else:
                    nc.scalar.copy(h_sb[:], h_ps[:])
                if lvl == 2:
                    ka = h_sb
                    continue

                # --- de-skew via DRAM bounce: R[j, k] = H[j, j + k]
                # Custom (non-slice) APs are invisible to Tile's dependency
                # tracker, so the read edges are added explicitly below.
                hd = hdpool.tile([128, HALO], FP32, tag="hd")
                hd_w = nc.sync.dma_start(hd[:], h_sb[:])
                if lvl == 3:
                    ka = h_sb
                    continue
                diag = bass.AP(hd[:].tensor, 0, [[HALO + 1, 128], [1, NK]])
                d_r = nc.sync.dma_start(rbig[:, rt * NK:(rt + 1) * NK], diag)
                add_dep_helper(d_r.ins, hd_w.ins, reason="deskew reads hd")
                diag_reads.append(d_r)
                if debug_taps:
                    nc.sync.dma_start(hdram[rt], hd[:])

              if lvl == 4:
                  ka = rbig
              if lvl < 5:
                  nc.sync.dma_start(out[0:1, 0:1], ka[0:1, 0:1])
                  return

              # --- column sums of rbig -> c-vectors, 3 matmuls of N=344
              csums = spool.tile([1, RT * NK], FP32, tag="csums")
              nchunk = RT * NK // 3        # 344
              cs_copies = []
              for i in range(3):
                  cs_ps = cps.tile([1, nchunk], FP32, tag="cs")
                  mm = nc.tensor.matmul(
                      cs_ps[:], ones128[:],
                      rbig[:, i * nchunk:(i + 1) * nchunk],
                      start=True, stop=True,
                  )
                  for d_r in diag_reads:
                      add_dep_helper(mm.ins, d_r.ins, reason="rbig ready")
                  cs_copies.append(
                      nc.scalar.copy(csums[:, i * nchunk:(i + 1) * nchunk],
                                     cs_ps[:]))

              if lvl < 6:
                  nc.sync.dma_start(out[0:1, 0:1], csums[0:1, 0:1])
                  return

              # --- scatter to [4, 2*129]: row b = [c_fake_b | c_real_b].
              # SBUF APs are partition-addressed, so the flat gather must go
              # through DRAM (flat byte addressing).
              cs_d = hdpool.tile([RT * NK], FP32, tag="csd")
              csd_w = nc.sync.dma_start(cs_d[:], csums[:])
              cs_mat = spool.tile([ROWS_PER_CORE, 2 * NK], FP32, tag="csmat")
              csrc = bass.AP(
                  cs_d[:].tensor, 0,
                  [[NK, ROWS_PER_CORE], [ROWS_PER_CORE * NK, 2], [1, NK]],
              )
              rearr = nc.sync.dma_start(cs_mat[:], csrc)
              add_dep_helper(rearr.ins, csd_w.ins, reason="cs_d ready")

              # --- normalize r = c / c0 (each half by its own c0)
              recf = spool.tile([ROWS_PER_CORE, 1], FP32, tag="recf")
              nc.vector.reciprocal(recf[:], cs_mat[:, 0:1])
              recr = spool.tile([ROWS_PER_CORE, 1], FP32, tag="recr")
              nc.vector.reciprocal(recr[:], cs_mat[:, NK:NK + 1])
              rf = spool.tile([ROWS_PER_CORE, NK], FP32, tag="rf")
              nc.vector.tensor_scalar_mul(rf[:], cs_mat[:, 0:NK], recf[:])
              rr = spool.tile([ROWS_PER_CORE, NK], FP32, tag="rr")
              nc.vector.tensor_scalar_mul(rr[:], cs_mat[:, NK:2 * NK], recr[:])

              # --- |r_fake - r_real| summed over k, per batch row
              diff = spool.tile([ROWS_PER_CORE, NK], FP32, tag="diff")
              nc.vector.tensor_sub(diff[:], rf[:], rr[:])
              absum = spool.tile([ROWS_PER_CORE, 1], FP32, tag="absum")
              nc.vector.tensor_reduce(
                  absum[:], diff[:], mybir.AxisListType.X, mybir.AluOpType.add,
                  apply_absolute_value=True,
              )
              nc.sync.dma_start(out[:], absum[:])
              if debug_taps:
                  nc.sync.dma_start(rdram[:], rbig[:])
                  nc.sync.dma_start(cdram[:], cs_mat[:])

            for _rep in range(reps):
                emit_rep()

    nc.compile()
    return nc


_CACHE = {}


def _get_program():
    if "nc" not in _CACHE:
        _CACHE["nc"] = build_program()
    return _CACHE["nc"]


def make_in_maps(fake: np.ndarray, real: np.ndarray):
    fake = np.asarray(fake, dtype=np.float32).reshape(B, L)
    real = np.asarray(real, dtype=np.float32).reshape(B, L)
    in_maps = []
    for c in range(N_CORES):
        rows = slice(c * ROWS_PER_CORE, (c + 1) * ROWS_PER_CORE)
        xin = np.concatenate([fake[rows], real[rows]], axis=0)
        in_maps.append({"xin": np.ascontiguousarray(xin)})
    return in_maps


def run(in_maps, **kwargs):
    """Run the SPMD program; returns (loss, BassKernelResults)."""
    res = run_bass_kernel_spmd(
        _get_program(), in_maps, list(range(N_CORES)), **kwargs
    )
    total = np.float64(0.0)
    for c in range(N_CORES):
        total += np.asarray(res.results[c]["out"], dtype=np.float64).sum()
    return np.float32(total / (B * NK)), res


def kernel(fake: np.ndarray, real: np.ndarray) -> np.ndarray:
    loss, _ = run(make_in_maps(fake, real))
    return loss



# revision 2
# speedup vs baseline: 1.5438x; 1.5438x over previous
"""AutoCorrelationLoss Trainium2 kernel (8-core SPMD, data-parallel over batch).

Math: for each row x (length L=8192), with com = L - 128 = 8064 = 128*63:
  ac[k] = mean(x0c * (Y_k - mean(Y_k)))  where x0c = x[:com] - mean(x[:com])
Since sum(x0c) = 0, the mean(Y_k) term vanishes:
  com * ac[k] = c[k] = sum_j x0c[j] * x[j+k]
Decompose j = 63*t + p (t<128, p<63) and let X2[t, f] = x[63t + f] (f<191),
W = X2[:, :63] - mean(x[:com]).  Then with H = W.T @ X2  ([63, 191]):
  c[k] = sum_j H[j, j+k]   (a skew sum over j<63, k = 0..128)
r[k] = c[k]/c[0];  loss = mean_{b,k} |r_fake - r_real|.

Per core: 4 batch rows x {fake, real} = 8 row-tensors, one bf16 matmul
[128,63]x[128,191] each; a diagonal-stride DMA through a DRAM bounce
de-skews H so the 129 skew-sums become column sums done by a ones-matmul.
All matmuls run in bf16 (1 cycle/row vs fp32's 4); fp32 PSUM accumulate
keeps the final scalar well inside the 2e-2 gate (measured ~5e-5).
"""

import sys

sys.path.insert(0, "/opt/trn_rl_repo")

import numpy as np

import concourse.bacc as bacc
import concourse.bass as bass
import concourse.mybir as mybir
import concourse.tile as tile
from concourse.bass_utils import run_bass_kernel_spmd
from concourse.tile_rust import add_dep_helper

B, L = 32, 8192
NCOEF = 128            # lags 0..128 -> 129 values
NK = NCOEF + 1         # 129
COM = L - NCOEF        # 8064 = 128 * 63
CH = 63                # contraction chunk width (free dim of weights)
NT = COM // CH         # 128 chunks -> full partition dim
HALO = CH + NCOEF      # 191
N_CORES = 8
ROWS_PER_CORE = B // N_CORES      # 4 batch rows per core
RT = 2 * ROWS_PER_CORE            # 8 row-tensors (fake rows then real rows)
HRT = RT // 2                     # rows per pipeline half

FP32 = mybir.dt.float32
BF16 = mybir.dt.bfloat16


def build_program():
    nc = bacc.Bacc(
        "TRN2",
        target_bir_lowering=False,
        debug=False,
        num_devices=N_CORES,
    )

    xin = nc.dram_tensor("xin", (RT, L), FP32, kind="ExternalInput")
    out = nc.dram_tensor("out", (1, 1), FP32, kind="ExternalOutput")

    with tile.TileContext(nc) as tc:
        with (
            tc.tile_pool(name="persist", bufs=1) as persist,
            tc.tile_pool(name="hdp", bufs=1, space=bass.MemorySpace.DRAM) as hdp,
            tc.tile_pool(name="hps", bufs=4, space=bass.MemorySpace.PSUM) as hps,
            tc.tile_pool(name="bps", bufs=2, space=bass.MemorySpace.PSUM) as bps,
            tc.tile_pool(name="cps", bufs=2, space=bass.MemorySpace.PSUM) as cps,
        ):
            ones_bf = persist.tile([NT, NT], BF16)      # partition-bcast weights
            nc.vector.memset(ones_bf[:], 1.0)
            ones_col = persist.tile([CH, 1], BF16)      # column-sum weights
            nc.vector.memset(ones_col[:], 1.0)

            xall = persist.tile([NT, RT, HALO], FP32)   # halo'd input
            xbf = persist.tile([NT, RT, HALO], BF16)    # bf16 rhs
            rowsums = persist.tile([NT, RT], FP32)
            msc = persist.tile([NT, RT], BF16)          # per-chunk means
            wbig = persist.tile([NT, RT, CH], BF16)     # centered weights
            hall = persist.tile([CH, RT, HALO], BF16)   # H matrices (SBUF)
            rbig = persist.tile([CH, RT, NK], BF16)     # de-skewed diagonals
            csums = persist.tile([1, RT * NK], FP32)    # c vectors, rt-major

            hd = hdp.tile([CH, RT, HALO], BF16)         # DRAM bounce for de-skew

            # --- two pipelined halves: load -> mean -> center -> matmul
            hd_writes = []
            for h in range(2):
                sl = slice(h * HRT, (h + 1) * HRT)
                eng = nc.sync if h == 0 else nc.scalar
                src = bass.AP(xin, h * HRT * L, [[CH, NT], [L, HRT], [1, HALO]])
                eng.dma_start(xall[:, sl, :], src)

                nc.vector.tensor_copy(xbf[:, sl, :], xall[:, sl, :])
                nc.vector.tensor_reduce(
                    rowsums[:, sl], xall[:, sl, 0:CH],
                    mybir.AxisListType.X, mybir.AluOpType.add,
                )
                nc.vector.tensor_scalar_mul(msc[:, sl], rowsums[:, sl], 1.0 / COM)
                # broadcast sum of per-chunk means (= row mean) over partitions
                mb = bps.tile([NT, HRT], FP32, tag="mb")
                nc.tensor.matmul(mb[:], ones_bf[:], msc[:, sl],
                                 start=True, stop=True)
                nc.vector.tensor_tensor(
                    wbig[:, sl, :], xall[:, sl, 0:CH],
                    mb[:].unsqueeze(2).broadcast_to([NT, HRT, CH]),
                    mybir.AluOpType.subtract,
                )
                for r in range(HRT):
                    rt = h * HRT + r
                    h_ps = hps.tile([CH, HALO], FP32, tag="h")
                    nc.tensor.matmul(h_ps[:], wbig[:, rt, :], xbf[:, rt, :],
                                     start=True, stop=True)
                    nc.vector.tensor_copy(hall[:, rt, :], h_ps[:])
                weng = nc.scalar if h == 0 else nc.sync
                hd_writes.append(weng.dma_start(hd[:, sl, :], hall[:, sl, :]))

            # --- de-skew via DRAM bounce: rbig[j, rt, k] = H_rt[j, j + k].
            # Custom (non-slice) APs are invisible to Tile's dependency
            # tracker, so the read/write edges are added explicitly.
            diag = bass.AP(hd[:].tensor, 0,
                           [[RT * HALO + 1, CH], [HALO, RT], [1, NK]])
            d_r = nc.sync.dma_start(rbig[:], diag)
            for hw_ in hd_writes:
                add_dep_helper(d_r.ins, hw_.ins, reason="deskew reads hd")

            # --- column sums of rbig -> c vectors, 3 matmuls of N=344
            rflat = rbig[:].rearrange("p a b -> p (a b)")
            nchunk = RT * NK // 3        # 344
            for i in range(3):
                cs_ps = cps.tile([1, nchunk], FP32, tag="cs")
                mm = nc.tensor.matmul(
                    cs_ps[:], ones_col[:],
                    rflat[:, i * nchunk:(i + 1) * nchunk],
                    start=True, stop=True,
                )
                add_dep_helper(mm.ins, d_r.ins, reason="rbig ready")
                nc.vector.tensor_copy(csums[:, i * nchunk:(i + 1) * nchunk],
                                      cs_ps[:])

            # --- r = c / c0 per rt, then sum_k |r_fake - r_real| on part. 0
            rec8 = persist.tile([1, RT], FP32)
            nc.vector.reciprocal(rec8[:], csums[0:1, 0:RT * NK:NK])
            rnorm = persist.tile([1, RT * NK], FP32)
            nc.vector.tensor_tensor(
                rnorm[:].rearrange("p (a b) -> p a b", a=RT),
                csums[:].rearrange("p (a b) -> p a b", a=RT),
                rec8[:].unsqueeze(2).broadcast_to([1, RT, NK]),
                mybir.AluOpType.mult,
            )
            half = ROWS_PER_CORE * NK    # 516
            diff = persist.tile([1, half], FP32)
            nc.vector.tensor_sub(diff[:], rnorm[0:1, 0:half],
                                 rnorm[0:1, half:2 * half])
            absum = persist.tile([1, 1], FP32)
            nc.vector.tensor_reduce(
                absum[:], diff[:], mybir.AxisListType.X, mybir.AluOpType.add,
                apply_absolute_value=True,
            )
            nc.sync.dma_start(out[0:1, 0:1], absum[:])

    nc.compile()
    return nc


_CACHE = {}


def _get_program():
    if "nc" not in _CACHE:
        _CACHE["nc"] = build_program()
    return _CACHE["nc"]


def make_in_maps(fake: np.ndarray, real: np.ndarray):
    fake = np.asarray(fake, dtype=np.float32).reshape(B, L)
    real = np.asarray(real, dtype=np.float32).reshape(B, L)
    in_maps = []
    for c in range(N_CORES):
        rows = slice(c * ROWS_PER_CORE, (c + 1) * ROWS_PER_CORE)
        xin = np.concatenate([fake[rows], real[rows]], axis=0)
        in_maps.append({"xin": np.ascontiguousarray(xin)})
    return in_maps


def run(in_maps, **kwargs):
    """Run the SPMD program; returns (loss, BassKernelResults)."""
    res = run_bass_kernel_spmd(
        _get_program(), in_maps, list(range(N_CORES)), **kwargs
    )
    total = np.float64(0.0)
    for c in range(N_CORES):
        total += np.float64(np.asarray(res.results[c]["out"]).reshape(()))
    return np.float32(total / (B * NK)), res


def kernel(fake: np.ndarray, real: np.ndarray) -> np.ndarray:
    loss, _ = run(make_in_maps(fake, real))
    return loss
